# revision 50
# baseline (speedup 1.0000x reference)
"""DeepseekV3 MoE kernel for 8 Trainium2 NeuronCores (expert-parallel).

Strategy:
  - Host: grouped top-k gating (exact replica of the reference jax ops, on CPU),
    token dispatch (gather tokens per expert, zero-padded to capacity C=128;
    spill rows beyond C run through an exact numpy fallback), and the
    final combine (scatter-add with top-k weights applied on host).
  - Device (SPMD over 8 cores): core c owns routed experts 8c..8c+7 and a
    64-wide slice of the intermediate dim of both shared experts.
    Routed expert weights ship as fp8 e3m4 scaled by 64 (PE upconverts to
    e10m11 internally, so all 4 mantissa bits are kept); activations and the
    shared experts ship as bf16. The x64 scale is undone in the silu
    (activation scale=1/64) and on the host (y carries a 64^2 factor).
  - Gate/up and down projections are software-pipelined across experts
    (down of expert e is emitted after gate of e+1) so the PE always has
    matmul work while the silu/h-mul chain drains on ACT/DVE.
  - Down-proj streams tokens (free dim C) with weights stationary, writing
    y in [128, HT, C] layout; the host un-transposes.
  - Host: scatter-add expert outputs back by token, sum shared partials.

Measured on HW: ~86-88 us/core (baseline fp32r version: 206-238 us),
rel err 7.6e-3 vs the fp32 reference (tolerance 2e-2).

Shapes (hardcoded): T=1024, H=1024, I=512, E=64, S=2, G=8, TOPK_GROUP=4, K=8.
"""
import numpy as np
import ml_dtypes
from contextlib import ExitStack

import concourse.bass as bass
from concourse import mybir, tile, bacc
from concourse.bass_utils import run_bass_kernel_spmd

f32 = mybir.dt.float32
bf16 = mybir.dt.bfloat16
fp8e3 = mybir.dt.float8e3
AF = mybir.ActivationFunctionType
nbf16 = ml_dtypes.bfloat16
ne3m4 = ml_dtypes.float8_e3m4

T, H, I, E, S = 1024, 1024, 512, 64, 2
G, TOPK_GROUP, K = 8, 4, 8
I2 = 2 * I
N_CORES = 8
E_LOC = E // N_CORES          # 8 experts per core
C = 128                       # per-expert token capacity (mean load 128, std ~11)
HT = H // 128                 # 8 k-tiles over hidden dim
IT = I // 128                 # 4 tiles over intermediate dim
ISH = I // N_CORES            # 64-wide shared-expert slice per core
WS = 64.0                     # fp8 weight pre-scale
M2 = I2 // 128                # 8 column blocks of gate_up weights

_TRACE = False
_CACHED_NC = None
LAST_RESULTS = None


def _build_nc():
    nc = bacc.Bacc("TRN2", target_bir_lowering=False, debug=False)

    # partition-major layouts: [..., 128, chunk, free] so each partition's
    # DRAM run is contiguous. Weights are chunked so the first matmul of the
    # kernel only waits on a 0.26 MB DMA, not a 2 MB one.
    xgt_d = nc.dram_tensor("xgt", [E_LOC, 128, HT, C], bf16, kind="ExternalInput")
    wgu_d = nc.dram_tensor("wgu", [E_LOC, 128, IT, HT, 256], fp8e3,
                           kind="ExternalInput")
    wd_d = nc.dram_tensor("wd", [E_LOC, 128, IT, H], fp8e3, kind="ExternalInput")
    xt_d = nc.dram_tensor("xt", [128, HT, T], bf16, kind="ExternalInput")
    swh_d = nc.dram_tensor("swh", [S, 128, HT, 2 * ISH], bf16, kind="ExternalInput")
    sdc_d = nc.dram_tensor("sdc", [S * ISH, H], bf16, kind="ExternalInput")
    y_d = nc.dram_tensor("y", [E_LOC, 128, HT, C], bf16, kind="ExternalOutput")
    sh_d = nc.dram_tensor("sh", [T, H], bf16, kind="ExternalOutput")

    with tile.TileContext(nc) as tc, ExitStack() as ctx:
        wgu_p = ctx.enter_context(tc.tile_pool(name="wgu", bufs=3))
        wd_p = ctx.enter_context(tc.tile_pool(name="wd", bufs=3))
        xgt_p = ctx.enter_context(tc.tile_pool(name="xgt", bufs=3))
        h_p = ctx.enter_context(tc.tile_pool(name="h", bufs=3))
        y_p = ctx.enter_context(tc.tile_pool(name="y", bufs=3))
        const_p = ctx.enter_context(tc.tile_pool(name="const", bufs=1))
        shh_p = ctx.enter_context(tc.tile_pool(name="shh", bufs=2))
        psA = ctx.enter_context(tc.tile_pool(name="psA", bufs=2, space="PSUM"))
        psB = ctx.enter_context(tc.tile_pool(name="psB", bufs=2, space="PSUM"))
        psC = ctx.enter_context(tc.tile_pool(name="psC", bufs=2, space="PSUM"))
        psD = ctx.enter_context(tc.tile_pool(name="psD", bufs=2, space="PSUM"))

        def emit_loads(e, issue_wd=True):
            xg = xgt_p.tile([128, HT, C], bf16, tag="xgt")
            wgF = wgu_p.tile([128, IT, HT, 256], fp8e3, tag="wguF")
            # expert 0 loads its first gate/up chunk and tokens in small
            # pieces so the first matmul starts as early as possible; later
            # experts are prefetched with one big DMA each to keep the SP
            # sequencer free.
            # token reads go through the scalar HWDGE ring so expert
            # weights stream back-to-back on the sync ring
            if e == 0:
                nc.sync.dma_start(wgF[:, 0, 0:HT // 2], wgu_d.ap()[e][:, 0, 0:HT // 2])
                nc.scalar.dma_start(xg[:, 0:HT // 2], xgt_d.ap()[e][:, 0:HT // 2])
                nc.sync.dma_start(wgF[:, 0, HT // 2:HT],
                                  wgu_d.ap()[e][:, 0, HT // 2:HT])
                nc.scalar.dma_start(xg[:, HT // 2:HT], xgt_d.ap()[e][:, HT // 2:HT])
                for it in range(1, IT):
                    nc.sync.dma_start(wgF[:, it], wgu_d.ap()[e][:, it])
            else:
                nc.scalar.dma_start(xg[:], xgt_d.ap()[e])
                nc.sync.dma_start(wgF[:], wgu_d.ap()[e])
            wdt = wd_p.tile([128, IT, H], fp8e3, tag="wd")
            if issue_wd:
                nc.sync.dma_start(wdt[:], wd_d.ap()[e])
            return xg, wgF, wdt

        def emit_gate(e, tiles):
            xg, wgF, wdt = tiles
            h_full = h_p.tile([128, IT, C], bf16, tag="h")
            for it in range(IT):
                ps = psA.tile([128, 2, C], f32, tag="gu")
                for h in range(HT):
                    nc.tensor.matmul(ps[:, 0, :], wgF[:, it, h, 0:128],
                                     xg[:, h, :],
                                     start=(h == 0), stop=(h == HT - 1))
                for h in range(HT):
                    nc.tensor.matmul(ps[:, 1, :], wgF[:, it, h, 128:256],
                                     xg[:, h, :],
                                     start=(h == 0), stop=(h == HT - 1))
                sl = h_p.tile([128, C], bf16, tag="silu")
                nc.scalar.activation(sl[:], ps[:, 0, :], AF.Silu, scale=1.0 / WS)
                nc.vector.tensor_mul(h_full[:, it, :], sl[:], ps[:, 1, :])
            return h_full

        def emit_down(e, tiles, h_full):
            xg, wgF, wdt = tiles
            yo = y_p.tile([128, HT, C], bf16, tag="y")
            last = e == E_LOC - 1
            seng = nc.scalar if last else nc.gpsimd
            for hp in range(HT // 2):
                ps_y = psB.tile([128, 2, C], f32, tag="dn")
                for sub in range(2):
                    hb = hp * 2 + sub
                    for it in range(IT):
                        nc.tensor.matmul(ps_y[:, sub, :],
                                         wdt[:, it, hb * 128:(hb + 1) * 128],
                                         h_full[:, it, :],
                                         start=(it == 0), stop=(it == IT - 1))
                    if hb % 2 == 0:
                        nc.vector.tensor_copy(yo[:, hb, :], ps_y[:, sub, :])
                    else:
                        nc.scalar.activation(yo[:, hb, :], ps_y[:, sub, :], AF.Copy)
                if last:
                    # drain the final expert in small pieces through BOTH
                    # HWDGE rings so the tail writes complete in parallel
                    weng = nc.scalar if hp % 2 == 0 else nc.sync
                    weng.dma_start(y_d.ap()[e][:, 2 * hp:2 * hp + 2],
                                   yo[:, 2 * hp:2 * hp + 2])
            if not last:
                seng.dma_start(y_d.ap()[e][:, 0:HT // 2], yo[:, 0:HT // 2])
                seng.dma_start(y_d.ap()[e][:, HT // 2:HT], yo[:, HT // 2:HT])

        hc_t = {}

        def emit_shared_const():
            xt = const_p.tile([128, HT, T], bf16, tag="xt")
            nc.sync.dma_start(xt[:], xt_d.ap()[:])
            swh_sb = []
            for s in range(S):
                sw = const_p.tile([128, HT, 2 * ISH], bf16, tag=f"swh{s}")
                nc.sync.dma_start(sw[:], swh_d.ap()[s])
                swh_sb.append(sw)
            sdc_sb = const_p.tile([S * ISH, H], bf16, tag="sdc")
            nc.sync.dma_start(sdc_sb[:], sdc_d.ap()[:])
            return xt, swh_sb, sdc_sb

        def emit_shared_gu(xt, swh_sb, tt):
            hc = shh_p.tile([128, 512], bf16, tag=f"hc{tt}")
            for s in range(S):
                ps = psC.tile([128, 512], f32, tag="big")
                for h in range(HT):
                    nc.tensor.matmul(ps[:], swh_sb[s][:, h, :],
                                     xt[:, h, tt * 512:(tt + 1) * 512],
                                     start=(h == 0), stop=(h == HT - 1))
                sl = shh_p.tile([ISH, 512], bf16, tag="slsh")
                nc.scalar.activation(sl[:], ps[0:ISH, :], AF.Silu)
                nc.vector.tensor_mul(hc[s * ISH:(s + 1) * ISH, :], sl[:],
                                     ps[ISH:2 * ISH, :])
            hc_t[tt] = hc

        def emit_shared_down(sdc_sb, tt):
            hc = hc_t[tt]
            for tp in range(4):
                so = y_p.tile([128, H], bf16, tag="so")
                for hh2 in range(2):
                    ps2 = psD.tile([128, 512], f32, tag="sdn")
                    nc.tensor.matmul(ps2[:], hc[:, tp * 128:(tp + 1) * 128],
                                     sdc_sb[:, hh2 * 512:(hh2 + 1) * 512],
                                     start=True, stop=True)
                    # 256-wide pieces so bulk copies don't head-of-line-block
                    # the latency-critical silu/h-mul chain on either engine
                    for q in range(2):
                        dst = so[:, hh2 * 512 + q * 256:hh2 * 512 + (q + 1) * 256]
                        nc.vector.tensor_copy(dst, ps2[:, q * 256:(q + 1) * 256])
                seng = nc.scalar if tp % 2 == 0 else nc.gpsimd
                seng.dma_start(
                    sh_d.ap()[(tt * 4 + tp) * 128:(tt * 4 + tp + 1) * 128, :],
                    so[:])

        tiles = [None] * E_LOC
        hf = [None] * E_LOC
        tiles[0] = emit_loads(0, issue_wd=False)
        tiles[1] = emit_loads(1)
        # e0's down weights are only needed after gate(1); issuing them here
        # lets e1's gate weights land ~1.3us earlier on the FIFO read queue
        nc.sync.dma_start(tiles[0][2][:], wd_d.ap()[0])
        hf[0] = emit_gate(0, tiles[0])
        tiles[2] = emit_loads(2)
        hf[1] = emit_gate(1, tiles[1])
        emit_down(0, tiles[0], hf[0])
        xt, swh_sb, sdc_sb = emit_shared_const()
        tiles[3] = emit_loads(3)
        hf[2] = emit_gate(2, tiles[2])
        emit_down(1, tiles[1], hf[1])
        tiles[4] = emit_loads(4)
        hf[3] = emit_gate(3, tiles[3])
        emit_down(2, tiles[2], hf[2])
        emit_shared_gu(xt, swh_sb, 0)
        tiles[5] = emit_loads(5)
        hf[4] = emit_gate(4, tiles[4])
        emit_down(3, tiles[3], hf[3])
        emit_shared_down(sdc_sb, 0)
        tiles[6] = emit_loads(6)
        hf[5] = emit_gate(5, tiles[5])
        emit_down(4, tiles[4], hf[4])
        emit_shared_gu(xt, swh_sb, 1)
        tiles[7] = emit_loads(7)
        hf[6] = emit_gate(6, tiles[6])
        emit_down(5, tiles[5], hf[5])
        emit_shared_down(sdc_sb, 1)
        hf[7] = emit_gate(7, tiles[7])
        emit_down(6, tiles[6], hf[6])
        emit_down(7, tiles[7], hf[7])
    nc.compile()
    return nc


def _route(x, gate_w):
    """Exact replica of the reference's grouped top-k gating, on CPU jax."""
    import jax
    import jax.numpy as jnp
    cpu = jax.devices("cpu")[0]
    with jax.default_device(cpu):
        xj = jax.device_put(np.asarray(x), cpu)
        gj = jax.device_put(np.asarray(gate_w), cpu)
        logits = xj @ gj.T
        t = logits.shape[0]
        group_size = E // G
        group_logits = logits.reshape(t, G, group_size)
        gw, gi = jax.lax.top_k(group_logits, TOPK_GROUP)
        gw = gw.reshape(t, G * TOPK_GROUP)
        gi = gi.reshape(t, G * TOPK_GROUP)
        topk_w, ti = jax.lax.top_k(gw, K)
        sel_group = ti // TOPK_GROUP
        expert_in_group = jnp.take_along_axis(gi, ti, axis=1)
        topk_idx = sel_group * group_size + expert_in_group
        topk_w = topk_w / (topk_w.sum(axis=-1, keepdims=True) + 1e-20)
    return np.asarray(topk_idx), np.asarray(topk_w).astype(np.float32)


def _expert_np(xrows, w_gu_e, w_d_e):
    """Reference expert math in numpy fp32 (overflow fallback only)."""
    g = xrows @ w_gu_e
    a = g[:, :I]
    hidden = (a / (1.0 + np.exp(-a))) * g[:, I:]
    return hidden @ w_d_e


def _to_e3m4(a):
    lim = 15.5
    return np.clip(a * WS, -lim, lim).astype(ne3m4)


def kernel(x, gate_w, w_gu, w_d, s_gu, s_d):
    global _CACHED_NC, LAST_RESULTS
    x = np.ascontiguousarray(np.asarray(x, dtype=np.float32))
    gate_w = np.ascontiguousarray(np.asarray(gate_w, dtype=np.float32))
    w_gu = np.ascontiguousarray(np.asarray(w_gu, dtype=np.float32))
    w_d = np.ascontiguousarray(np.asarray(w_d, dtype=np.float32))
    s_gu = np.ascontiguousarray(np.asarray(s_gu, dtype=np.float32))
    s_d = np.ascontiguousarray(np.asarray(s_d, dtype=np.float32))

    topk_idx, topk_w = _route(x, gate_w)

    flat_e = topk_idx.ravel()
    flat_t = np.repeat(np.arange(T), K)
    flat_w = topk_w.ravel()
    order = np.argsort(flat_e, kind="stable")
    sorted_t = flat_t[order]
    sorted_w = flat_w[order]
    counts = np.bincount(flat_e, minlength=E)
    starts = np.zeros(E + 1, np.int64)
    np.cumsum(counts, out=starts[1:])

    xT = np.ascontiguousarray(x.T)  # [H, T]
    xgt = np.zeros((E, H, C), np.float32)
    overflow = []
    for e in range(E):
        n = int(counts[e])
        toks = sorted_t[starts[e]:starts[e] + n]
        nn = min(n, C)
        xgt[e, :, :nn] = xT[:, toks[:nn]]
        if n > C:
            ws = sorted_w[starts[e]:starts[e] + n]
            overflow.append((e, toks[C:], ws[C:]))

    # partition-major shuffles for big DMA packets
    xgt_s = np.ascontiguousarray(
        xgt.reshape(E, HT, 128, C).transpose(0, 2, 1, 3).astype(nbf16))
    # wgu_s[e, p, it, ht, 0:128|128:256] = gate block it | up block it (e3m4 x64)
    wq = _to_e3m4(w_gu)
    wgu_s = np.ascontiguousarray(np.concatenate(
        [wq[:, :, :I].reshape(E, HT, 128, IT, 128),
         wq[:, :, I:].reshape(E, HT, 128, IT, 128)],
        axis=4).transpose(0, 2, 3, 1, 4))
    wd_s = np.ascontiguousarray(
        _to_e3m4(w_d).reshape(E, IT, 128, H).transpose(0, 2, 1, 3))
    xt_s = np.ascontiguousarray(
        xT.reshape(HT, 128, T).transpose(1, 0, 2).astype(nbf16))  # [128,HT,T]

    if _CACHED_NC is None:
        _CACHED_NC = _build_nc()
    nc = _CACHED_NC

    in_maps = []
    for c in range(N_CORES):
        lo = c * E_LOC
        sl = slice(c * ISH, (c + 1) * ISH)
        swh = np.concatenate([s_gu[:, :, sl], s_gu[:, :, I:][:, :, sl]], axis=2)
        swh_s = np.ascontiguousarray(
            swh.reshape(S, HT, 128, 2 * ISH).transpose(0, 2, 1, 3).astype(nbf16))
        sdc = np.ascontiguousarray(
            s_d[:, sl, :].reshape(S * ISH, H).astype(nbf16))
        in_maps.append({
            "xgt": xgt_s[lo:lo + E_LOC],
            "wgu": wgu_s[lo:lo + E_LOC],
            "wd": wd_s[lo:lo + E_LOC],
            "xt": xt_s,
            "swh": swh_s,
            "sdc": sdc,
        })

    res = run_bass_kernel_spmd(nc, in_maps, list(range(N_CORES)), trace=_TRACE)
    LAST_RESULTS = res

    out = np.zeros((T, H), np.float32)
    for c in range(N_CORES):
        out += res.results[c]["sh"].astype(np.float32)

    # y arrives [E_LOC, 128, HT, C] carrying a WS^2 factor -> [E, C, H] fp32
    y_all = np.concatenate(
        [np.asarray(res.results[c]["y"]).astype(np.float32)
         .transpose(0, 3, 2, 1).reshape(E_LOC, C, H)
         for c in range(N_CORES)], axis=0) * (1.0 / (WS * WS))
    routed_rows = np.empty((T * K, H), np.float32)
    pos = 0
    tok_order = np.empty(T * K, np.int64)
    for e in range(E):
        n = min(int(counts[e]), C)
        ws = sorted_w[starts[e]:starts[e] + n]
        routed_rows[pos:pos + n] = y_all[e, :n] * ws[:, None]
        tok_order[pos:pos + n] = sorted_t[starts[e]:starts[e] + n]
        pos += n
    inv = np.argsort(tok_order[:pos], kind="stable")
    if pos == T * K:
        routed = routed_rows[inv].reshape(T, K, H).sum(axis=1)
        out += routed
    else:
        np.add.at(out, tok_order[:pos][inv], routed_rows[:pos][inv])

    for e, toks, ws in overflow:
        y_extra = _expert_np(x[toks], w_gu[e], w_d[e]) * ws[:, None]
        np.add.at(out, toks, y_extra)

    return out
